# revision 11
# baseline (speedup 1.0000x reference)
"""Trainium2 Bass kernel for the BDH-style recurrent block.

Strategy: data-parallel over B (8 batches -> 8 NeuronCores, no collectives).
The T=128-step scan is de-sequentialized into dense matmuls per core:

  u_t = relu(emb_t @ Dx.T)                                  (T,N) batched matmul
  x_t = (XD*x_{t-1} + u_t)/s_t  with s_t = XD + sum(u_t)    (L1 norm; x>=0)
      => x = C @ u, C[t,s] = (1/s_s) exp(A_t - A_s), A_t = cumsum log(XD/s_r)
  a*_t = rho_{t-1} @ x_t = ((DecayMask . X X^T) @ ln(emb))_t   (rho_0 = 0)
  y_t  = relu(ln(a*_t) @ Dy.T) * x_t                        (x_t >= 0)
  v*_t = ln(y_t @ E.T)

Matmuls run in float32r (~1.5e-4 rounding, 4x faster PE streaming at free
dims >= 256). X/Y live in (t, n)-major layout; PE transposes provide the
n-major tiles needed for the Gram matrix and the E contraction. A bf16
dummy-matmul warmup during the initial weight DMA lifts the PE HAM clock
gate to 2.4 GHz before the real work arrives.
"""

import math
from contextlib import ExitStack

import numpy as np

N = 2048
D = 256
B = 8
T = 128
XD = 0.97
UD = 0.97
LN_EPS = 1e-5
L1_EPS = 1e-12

# log-domain recentring: E[sum relu(N(0,1)) over 2048] + XD ~ 818.9
LNC2INV = 6.7065
C2 = math.exp(-LNC2INV)
K1 = LNC2INV - math.log(XD)

KD = D // 128   # 2
KN = N // 128   # 16
NJ = N // 512   # 4
WARMUP_MMS = 48

_cache = {}


def _consts():
    r = np.arange(T)
    utones = (r[:, None] <= r[None, :]).astype(np.float32)          # [r,t] r<=t
    negones = -np.ones((1, T), dtype=np.float32)
    tri = r[None, :] - r[:, None]                                   # t - s
    trik = np.where(tri >= 0, -K1 * tri - LNC2INV, -10000.0).astype(np.float32)
    pw = r[:, None] - 1 - r[None, :]                                # [t,s] t-1-s
    dmask = np.where(pw >= 0, UD ** np.maximum(pw, 0), 0.0).astype(np.float32)
    dmaskT = np.ascontiguousarray(dmask.T)                          # [s,t]
    ident = np.eye(T, dtype=np.float32)
    xdvec = np.full((T, 1), C2 * XD, dtype=np.float32)
    xdvec[0, 0] = 0.0                                               # x_{-1} = 0
    return utones, negones, trik, dmaskT, ident, xdvec


def _split_multiwait(nc, mybir):
    """This walrus build caps sync waits per instruction (1 for regular
    instructions, 2 for EventSemaphore). Tile attaches more (e.g. the
    kernel-tail Drain waits on every live semaphore). Hoist excess waits
    onto same-engine NOPs placed immediately before the instruction —
    engine queues are sequential, so semantics are preserved."""
    n = 0
    for f in nc.m.functions:
        for bb in f.blocks:
            out = []
            changed = False
            for ins in bb.instructions:
                si = ins.sync_info
                ow = list(si.on_wait) if si is not None else []
                cap = 2 if ins.opcode == "EventSemaphore" else 1
                if len(ow) > cap:
                    sem_waits = [w for w in ow if w.sync_type == "semaphore"]
                    other = [w for w in ow if w.sync_type != "semaphore"]
                    keep = max(cap - len(other), 0)
                    hoist = sem_waits[:len(sem_waits) - keep] if keep else sem_waits
                    kept = sem_waits[len(hoist):] + other
                    assert len(kept) <= cap, (len(kept), cap, ins.opcode)
                    changed = True
                    for w in hoist:
                        n += 1
                        nop = mybir.InstNoOp(
                            name=f"wsplit-{n}",
                            sync_info=mybir.SyncInfo(on_wait=[w], on_update=[]),
                            bass_nofuse=True,
                            engine=ins.engine,
                        )
                        nc.register_instruction(nop, overwrite=True)
                        out.append(nop)
                    si.on_wait = kept
                out.append(ins)
            if changed:
                bb.instructions = out
    return nc


def _build():
    import concourse.bass as bass
    import concourse.mybir as mybir
    import concourse.tile as tile

    f32 = mybir.dt.float32
    f32r = mybir.dt.float32r
    bf16 = mybir.dt.bfloat16
    AF = mybir.ActivationFunctionType
    ALU = mybir.AluOpType
    AX = mybir.AxisListType

    nc = bass.Bass()

    d_emb = nc.dram_tensor("emb", [T, D], f32, kind="ExternalInput")
    d_embT = nc.dram_tensor("embT", [KD, 128, T], f32, kind="ExternalInput")
    d_dxT = nc.dram_tensor("dxT", [KD, 128, N], f32, kind="ExternalInput")
    d_dyT = nc.dram_tensor("dyT", [KD, 128, N], f32, kind="ExternalInput")
    d_eT = nc.dram_tensor("eT", [KN, 128, D], f32, kind="ExternalInput")
    d_utones = nc.dram_tensor("utones", [T, T], f32, kind="ExternalInput")
    d_negones = nc.dram_tensor("negones", [1, T], f32, kind="ExternalInput")
    d_trik = nc.dram_tensor("trik", [T, T], f32, kind="ExternalInput")
    d_dmaskT = nc.dram_tensor("dmaskT", [T, T], f32, kind="ExternalInput")
    d_ident = nc.dram_tensor("ident", [T, T], f32, kind="ExternalInput")
    d_xdvec = nc.dram_tensor("xdvec", [T, 1], f32, kind="ExternalInput")
    d_out = nc.dram_tensor("out", [T, D], f32, kind="ExternalOutput")

    with tile.TileContext(nc) as tc, ExitStack() as ctx:
        work = ctx.enter_context(tc.tile_pool(name="work", bufs=1))
        stats = ctx.enter_context(tc.tile_pool(name="stats", bufs=1))
        p_u = ctx.enter_context(tc.tile_pool(name="p_u", bufs=2, space="PSUM"))
        p_sq = ctx.enter_context(tc.tile_pool(name="p_sq", bufs=4, space="PSUM"))
        p_g = ctx.enter_context(tc.tile_pool(name="p_g", bufs=1, space="PSUM"))
        p_med = ctx.enter_context(tc.tile_pool(name="p_med", bufs=1, space="PSUM"))

        # ---- PE warmup: bf16 dummy matmuls while weights stream in ------
        wu_sb = work.tile([128, 128], bf16)
        nc.vector.memset(wu_sb[:], 0.0)
        wu_ps = p_sq.tile([T, T], f32, tag="sq")
        for i in range(WARMUP_MMS):
            nc.tensor.matmul(wu_ps[:], wu_sb[:], wu_sb[:], start=True, stop=True)

        # ---- activation table preloads (Ln/Exp used mid-kernel) ---------
        pre_sb = stats.tile([1, 1], f32)
        nc.vector.memset(pre_sb[:], 1.0)
        pre_o = stats.tile([1, 1], f32)
        nc.scalar.activation(pre_o[:], pre_sb[:], AF.Ln)
        nc.scalar.activation(pre_o[:], pre_sb[:], AF.Exp)
        nc.scalar.activation(pre_o[:], pre_sb[:], AF.Square)

        # ---- DMAs (dxT first: it gates the first real matmuls) ----------
        dxT_sb = work.tile([128, KD * N], f32r)
        for c in range(KD):
            for j in range(NJ):
                nc.sync.dma_start(
                    dxT_sb[:, c * N + j * 512: c * N + (j + 1) * 512],
                    d_dxT[c][:, j * 512:(j + 1) * 512].bitcast(f32r),
                )
        embT_sb = work.tile([128, KD * T], f32r)
        for c in range(KD):
            nc.sync.dma_start(embT_sb[:, c * T:(c + 1) * T], d_embT[c].bitcast(f32r))
        emb_sb = work.tile([T, D], f32)
        nc.sync.dma_start(emb_sb[:], d_emb[:])
        utones_sb = work.tile([T, T], f32)
        nc.sync.dma_start(utones_sb[:], d_utones[:])
        negones_sb = work.tile([1, T], f32)
        nc.sync.dma_start(negones_sb[:], d_negones[:])
        trik_sb = work.tile([T, T], f32)
        nc.sync.dma_start(trik_sb[:], d_trik[:])
        dmaskT_sb = work.tile([T, T], f32)
        nc.sync.dma_start(dmaskT_sb[:], d_dmaskT[:])
        ident_sb = work.tile([T, T], f32r)
        nc.sync.dma_start(ident_sb[:], d_ident[:].bitcast(f32r))
        xdvec_sb = stats.tile([T, 1], f32)
        nc.sync.dma_start(xdvec_sb[:], d_xdvec[:])
        dyT_sb = work.tile([128, KD * N], f32r)
        for c in range(KD):
            for j in range(2):
                nc.sync.dma_start(
                    dyT_sb[:, c * N + j * 1024: c * N + (j + 1) * 1024],
                    d_dyT[c][:, j * 1024:(j + 1) * 1024].bitcast(f32r),
                )
        eT_sb = work.tile([128, KN * D], f32r)
        for c in range(KN):
            nc.sync.dma_start(eT_sb[:, c * D:(c + 1) * D], d_eT[c].bitcast(f32r))

        # ---- all-ACT layernorm helper -----------------------------------
        def layernorm(src, dst, tagp, evac=None, msum_src=None):
            """dst = LN(src) over free dim. If evac is a PSUM AP, src is
            filled from it (evac+rowsum fused); else src must be SBUF and
            a junk copy produces the rowsum."""
            junk = work.tile([T, D], f32, tag="lnjunk")
            msum = stats.tile([T, 1], f32, tag=f"{tagp}_ms")
            if evac is not None:
                nc.scalar.activation(src[:], evac, AF.Copy, accum_out=msum[:])
            else:
                nc.scalar.activation(junk[:], src[:], AF.Copy, accum_out=msum[:])
            negm = stats.tile([T, 1], f32, tag=f"{tagp}_nm")
            nc.scalar.mul(negm[:], msum[:], -1.0 / D)
            ssum = stats.tile([T, 1], f32, tag=f"{tagp}_ss")
            nc.scalar.activation(junk[:], src[:], AF.Square, bias=negm[:],
                                 accum_out=ssum[:])
            veps = stats.tile([T, 1], f32, tag=f"{tagp}_ve")
            nc.vector.tensor_scalar(veps[:], ssum[:], 1.0 / D, LN_EPS,
                                    op0=ALU.mult, op1=ALU.add)
            lv = stats.tile([T, 1], f32, tag=f"{tagp}_lv")
            nc.scalar.activation(lv[:], veps[:], AF.Ln)
            rstd = stats.tile([T, 1], f32, tag=f"{tagp}_rs")
            nc.scalar.activation(rstd[:], lv[:], AF.Exp, scale=-0.5)
            nmr = stats.tile([T, 1], f32, tag=f"{tagp}_nr")
            nc.scalar.mul(nmr[:], negm[:], rstd[:])
            nc.scalar.activation(dst[:], src[:], AF.Identity,
                                 scale=rstd[:], bias=nmr[:])

        # ---- u = relu(emb @ Dx.T) (f32r), row sums ----------------------
        u_sb = work.tile([T, N], f32r)
        su_part = stats.tile([T, NJ], f32)
        for j in range(NJ):
            ps = p_u.tile([128, 512], f32, tag="pu")
            for c in range(KD):
                nc.tensor.matmul(
                    ps[:],
                    embT_sb[:, c * T:(c + 1) * T],
                    dxT_sb[:, c * N + j * 512: c * N + (j + 1) * 512],
                    start=(c == 0),
                    stop=(c == KD - 1),
                )
            nc.scalar.activation(
                u_sb[:, j * 512:(j + 1) * 512], ps[:], AF.Relu,
                accum_out=su_part[:, j:j + 1],
            )

        # ---- C^T coefficient matrix -------------------------------------
        su = stats.tile([T, 1], f32)
        nc.vector.tensor_reduce(su[:], su_part[:], axis=AX.X, op=ALU.add)
        q_sb = stats.tile([T, 1], f32)
        nc.scalar.activation(q_sb[:], su[:], AF.Ln, scale=C2, bias=xdvec_sb[:])

        qc = p_sq.tile([T, T], f32, tag="sq")               # Q_s column
        nc.tensor.matmul(qc[:, 0:1], utones_sb[:], q_sb[:], start=True, stop=True)
        qr = p_sq.tile([T, T], f32, tag="sq")               # Q_t row
        nc.tensor.matmul(qr[0:1, :], q_sb[:], utones_sb[:], start=True, stop=True)
        qr_sb = stats.tile([1, T], f32)
        nc.vector.tensor_copy(qr_sb[:], qr[0:1, :])
        colsc = stats.tile([T, 1], f32)                     # Q_s - q_s
        nc.vector.tensor_sub(colsc[:], qc[:, 0:1], q_sb[:])
        bc = p_sq.tile([T, T], f32, tag="sq")               # [s,t] = -Q_t
        nc.tensor.matmul(bc[:], negones_sb[:], qr_sb[:], start=True, stop=True)

        expo = work.tile([T, T], f32)
        nc.vector.scalar_tensor_tensor(
            expo[:], bc[:], colsc[:], trik_sb[:], op0=ALU.add, op1=ALU.add
        )
        expoc = work.tile([T, T], f32)
        nc.vector.tensor_scalar_max(expoc[:], expo[:], -87.0)
        ct_sb = work.tile([T, T], f32r)                     # C^T [s,t]
        nc.scalar.activation(ct_sb[:], expoc[:], AF.Exp)

        # ---- vn = LN(emb) (off critical path) ---------------------------
        vn_sb = work.tile([T, D], f32r)
        layernorm(emb_sb, vn_sb, "vn")

        # ---- X = C @ u (t,n-major, f32r), X^T via PE transpose ----------
        x_sb = work.tile([T, N], f32r)
        for j in range(NJ):
            ps = p_u.tile([128, 512], f32, tag="pu")
            nc.tensor.matmul(ps[:], ct_sb[:], u_sb[:, j * 512:(j + 1) * 512],
                             start=True, stop=True)
            if j % 2 == 0:
                nc.vector.tensor_copy(x_sb[:, j * 512:(j + 1) * 512], ps[:])
            else:
                nc.scalar.copy(x_sb[:, j * 512:(j + 1) * 512], ps[:])

        xt_sb = work.tile([128, N], f32r)
        for c in range(KN):
            tp = p_sq.tile([T, T], f32, tag="sq")
            nc.tensor.transpose(tp[:].bitcast(f32r), x_sb[:, c * T:(c + 1) * T],
                                ident_sb[:])
            if c % 2 == 0:
                nc.vector.tensor_copy(xt_sb[:, c * T:(c + 1) * T], tp[:])
            else:
                nc.scalar.copy(xt_sb[:, c * T:(c + 1) * T], tp[:])

        # ---- G = X X^T, W^T = G . mask ----------------------------------
        g = p_g.tile([T, T], f32, tag="g")
        for c in range(KN):
            nc.tensor.matmul(g[:], xt_sb[:, c * T:(c + 1) * T],
                             xt_sb[:, c * T:(c + 1) * T],
                             start=(c == 0), stop=(c == KN - 1))
        wt_sb = work.tile([T, T], f32r)
        nc.vector.tensor_mul(wt_sb[:], g[:], dmaskT_sb[:])

        # ---- a* = W @ vn, LN, transpose ---------------------------------
        aps = p_med.tile([T, D], f32, tag="med")
        nc.tensor.matmul(aps[:], wt_sb[:], vn_sb[:], start=True, stop=True)
        astar_sb = work.tile([T, D], f32)
        lna_sb = work.tile([T, D], f32r)
        layernorm(astar_sb, lna_sb, "la", evac=aps[:])

        lnaT_sb = work.tile([128, KD * T], f32r)
        for c in range(KD):
            tp = p_sq.tile([T, T], f32, tag="sq")
            nc.tensor.transpose(tp[:].bitcast(f32r), lna_sb[:, c * T:(c + 1) * T],
                                ident_sb[:])
            nc.scalar.copy(lnaT_sb[:, c * T:(c + 1) * T], tp[:])

        # ---- Ycore (t,n-major, f32r), Y = relu(Ycore) * X ---------------
        y_sb = work.tile([T, N], f32r)
        for j in range(NJ):
            ps = p_u.tile([128, 512], f32, tag="pu")
            for k in range(KD):
                nc.tensor.matmul(ps[:], lnaT_sb[:, k * T:(k + 1) * T],
                                 dyT_sb[:, k * N + j * 512: k * N + (j + 1) * 512],
                                 start=(k == 0), stop=(k == KD - 1))
            nc.vector.scalar_tensor_tensor(
                y_sb[:, j * 512:(j + 1) * 512], ps[:], 0.0,
                x_sb[:, j * 512:(j + 1) * 512], op0=ALU.max, op1=ALU.mult,
            )

        # ---- Y^T via transpose, v_raw = Y @ E.T -------------------------
        yt_sb = work.tile([128, N], f32r)
        for c in range(KN):
            tp = p_sq.tile([T, T], f32, tag="sq")
            nc.tensor.transpose(tp[:].bitcast(f32r), y_sb[:, c * T:(c + 1) * T],
                                ident_sb[:])
            if c % 2 == 0:
                nc.vector.tensor_copy(yt_sb[:, c * T:(c + 1) * T], tp[:])
            else:
                nc.scalar.copy(yt_sb[:, c * T:(c + 1) * T], tp[:])

        vps = p_med.tile([T, D], f32, tag="med")
        for c in range(KN):
            nc.tensor.matmul(vps[:], yt_sb[:, c * T:(c + 1) * T],
                             eT_sb[:, c * D:(c + 1) * D],
                             start=(c == 0), stop=(c == KN - 1))
        vraw_sb = work.tile([T, D], f32)
        vstar_sb = work.tile([T, D], f32)
        layernorm(vraw_sb, vstar_sb, "vs", evac=vps[:])

        nc.sync.dma_start(d_out[:], vstar_sb[:])

    return _split_multiwait(nc, mybir)


def _numpy_fallback(embeddings, E, Dx, Dy, x_state, rho_state):
    # General-path reference (only used if initial states are nonzero).
    def ln(x):
        m = x.mean(-1, keepdims=True)
        v = ((x - m) ** 2).mean(-1, keepdims=True)
        return (x - m) / np.sqrt(v + LN_EPS)

    x_s = x_state.astype(np.float32).copy()
    rho = rho_state.astype(np.float32).copy()
    outs = np.zeros((B, T, D), dtype=np.float32)
    for t in range(T):
        v_prev = embeddings[:, t, :]
        x_upd = np.maximum(v_prev @ Dx.T, 0.0)
        x_t = XD * x_s + x_upd
        x_t = x_t / np.maximum(np.abs(x_t).sum(-1, keepdims=True), L1_EPS)
        a_star = np.einsum("bdn,bn->bd", rho, x_t)
        y_core = ln(a_star) @ Dy.T
        y_t = np.maximum(y_core, 0.0) * np.maximum(x_t, 0.0)
        outs[:, t, :] = ln(y_t @ E.T)
        vn = ln(v_prev)
        rho = UD * rho + np.einsum("bd,bn->bdn", vn, x_t)
        x_s = x_t
    return outs


def kernel(embeddings, E, Dx, Dy, x_state, rho_state):
    embeddings = np.ascontiguousarray(embeddings, dtype=np.float32)
    E = np.ascontiguousarray(E, dtype=np.float32)
    Dx = np.ascontiguousarray(Dx, dtype=np.float32)
    Dy = np.ascontiguousarray(Dy, dtype=np.float32)

    if np.any(x_state) or np.any(rho_state):
        return _numpy_fallback(embeddings, E, Dx, Dy,
                               np.asarray(x_state, np.float32),
                               np.asarray(rho_state, np.float32))

    from concourse.bass_utils import run_bass_kernel_spmd

    if "nc" not in _cache:
        _cache["nc"] = _build()
    nc = _cache["nc"]

    utones, negones, trik, dmaskT, ident, xdvec = _consts()
    dxT = np.ascontiguousarray(Dx.T).reshape(KD, 128, N)
    dyT = np.ascontiguousarray(Dy.T).reshape(KD, 128, N)
    eT = np.ascontiguousarray(E.T).reshape(KN, 128, D)

    in_maps = []
    for b in range(B):
        emb_b = embeddings[b]
        in_maps.append({
            "emb": emb_b,
            "embT": np.ascontiguousarray(emb_b.T).reshape(KD, 128, T),
            "dxT": dxT,
            "dyT": dyT,
            "eT": eT,
            "utones": utones,
            "negones": negones,
            "trik": trik,
            "dmaskT": dmaskT,
            "ident": ident,
            "xdvec": xdvec,
        })

    res = run_bass_kernel_spmd(nc, in_maps, list(range(B)))
    _cache["last_results"] = res
    return np.stack([res.results[i]["out"] for i in range(B)])
